# revision 24
# baseline (speedup 1.0000x reference)
"""Tensor-parallel Llama GQA attention layer (B=1, S=2048, D=2048, H=32, KV=8)
for 8 Trainium2 NeuronCores.

Sharding: one KV group per core (kv head g + its 4 q heads). Each core computes
its heads' attention and a partial out-projection (contraction over its 256
head-dim columns of wo); the host sums the 8 bf16 partials (the TP all-reduce)
and transposes back to [1, S, D].

On-core layout is feature-major (transposed): xt=[D,S], QT=[j,S], KT/VT=[hd,S].
Scores are built per (head-pair, s-superblock of 512, t-block of 128) as
ST=[t,s] tiles; softmax is unnormalized exp with the denominator from a
ones-column appended to V, one normalization divide at the end.

v2 structure (vs the 243us baseline):
- input DMAs ordered by first use (ident/wkv/xt-chunk0 first) and xt chunk 0
  split in two so the KV projection starts ~5us in; ~32 tiny identity matmuls
  warm the PE HAM clock during the DMA lead-in (cold K=4/8 halves PE clock).
- per-superblock software pipeline: attention(si) is emitted interleaved with
  the projection of chunk si+1 and the out-projection of superblock si-1 as
  PE filler, so the PE never idles while ScalarE chews exp (the HAM
  oscillation in the baseline cost ~34us of half-clock time).
- causal trimming at 128-col granularity: for diagonal t-blocks the scores
  matmul, exp, and AV matmul only cover columns >= the block start (PSUM
  per-element has_written bits make partially-covering accumulation groups
  correct); the mask is a single [128,128] lower-tri multiply per diag block.
- output partials in bf16 (halves the 16.8MB output DMA).
"""

import numpy as np
import ml_dtypes

S = 2048
D = 2048
H = 32
KV = 8
HD = 64
R = 4  # heads per kv group
NC = 8  # cores

BF16 = ml_dtypes.bfloat16


def _build_program():
    import concourse.mybir as mybir
    import concourse.tile as tile
    from concourse import bacc

    f32 = mybir.dt.float32
    bf16 = mybir.dt.bfloat16

    nc = bacc.Bacc("TRN2", debug=False, num_devices=NC)

    xt = nc.dram_tensor("xt", [D, S], bf16, kind="ExternalInput")
    wq_t = nc.dram_tensor("wq_t", [D, R * HD], bf16, kind="ExternalInput")
    wkv_t = nc.dram_tensor("wkv_t", [D, 2 * HD], bf16, kind="ExternalInput")
    wo_t = nc.dram_tensor("wo_t", [R * HD, D], bf16, kind="ExternalInput")
    cosb = nc.dram_tensor("cosb", [128, S], bf16, kind="ExternalInput")
    sinb = nc.dram_tensor("sinb", [128, S], bf16, kind="ExternalInput")
    tri = nc.dram_tensor("tri", [128, 128], bf16, kind="ExternalInput")
    ident64 = nc.dram_tensor("ident64", [64, 64], bf16, kind="ExternalInput")
    out_t = nc.dram_tensor("out_t", [D, S], bf16, kind="ExternalOutput")

    DT = D // 128  # 16 d tiles
    SB = S // 512  # 4 s superblocks

    with tile.TileContext(nc) as tc:
        with (
            tc.tile_pool(name="persist", bufs=1) as persist,
            tc.tile_pool(name="qstage", bufs=2) as qstage_p,
            tc.tile_pool(name="rtmp", bufs=2) as rtmp_p,
            tc.tile_pool(name="et", bufs=6) as etp,
            tc.tile_pool(name="norm", bufs=4) as normp,
            tc.tile_pool(name="ostage", bufs=3) as ostage_p,
            tc.tile_pool(name="qkv_ps", bufs=2, space="PSUM") as qkv_ps,
            tc.tile_pool(name="st_ps", bufs=2, space="PSUM") as st_ps,
            tc.tile_pool(name="ut_ps", bufs=2, space="PSUM") as ut_ps,
        ):
            # ---- persistent SBUF tensors + input DMA (ordered by first use) --
            xt_sb = persist.tile([128, DT, S], bf16)
            wq_sb = persist.tile([128, DT, R * HD], bf16)
            wkv_sb = persist.tile([128, DT, 2 * HD], bf16)
            wo_sb = persist.tile([128, 2, D], bf16)
            cos_sb = persist.tile([128, S], bf16)
            sin_sb = persist.tile([128, S], bf16)
            tri_sb = persist.tile([128, 128], bf16)
            ident_sb = persist.tile([128, 64], bf16)
            wkv_r = wkv_t.ap().rearrange("(dt p) j -> p dt j", p=128)
            wq_r = wq_t.ap().rearrange("(dt p) j -> p dt j", p=128)
            xt_r = xt.ap().rearrange("(dt p) s -> p dt s", p=128)
            # bulk activations/weights on the sync queue, ordered by first use;
            # small constants on the (otherwise idle) gpsimd queue so they
            # don't delay the xt stream.
            # weights go down the gpsimd queue concurrently with the xt
            # stream on the sync queue — each queue has its own ramp-up, so
            # two queues roughly halve the time to first-compute.
            nc.gpsimd.dma_start(out=wkv_sb, in_=wkv_r)
            nc.sync.dma_start(out=xt_sb[:, 0:4, 0:512], in_=xt_r[:, 0:4, 0:512])
            nc.gpsimd.dma_start(out=wq_sb, in_=wq_r)
            nc.sync.dma_start(out=xt_sb[:, 4:8, 0:512], in_=xt_r[:, 4:8, 0:512])
            nc.sync.dma_start(out=xt_sb[:, 8:16, 0:512], in_=xt_r[:, 8:16, 0:512])
            nc.gpsimd.dma_start(out=ident_sb[64:128, :], in_=ident64.ap())
            nc.gpsimd.dma_start(out=cos_sb, in_=cosb.ap())
            nc.gpsimd.dma_start(out=sin_sb, in_=sinb.ap())
            nc.gpsimd.dma_start(out=tri_sb, in_=tri.ap())
            for sc in range(1, 4):
                nc.sync.dma_start(
                    out=xt_sb[:, :, sc * 512:(sc + 1) * 512],
                    in_=xt_r[:, :, sc * 512:(sc + 1) * 512])
            for jt in range(2):
                nc.sync.dma_start(out=wo_sb[:, jt, :], in_=wo_t.ap()[jt * 128:(jt + 1) * 128, :])

            qtr_sb = persist.tile([128, 2, S], bf16)   # roped Q, head-major
            kv2_sb = persist.tile([128, S], bf16)      # 0:64 roped K, 64:128 VT
            ko_sb = persist.tile([128, S], bf16)       # 64:128 roped K (odd heads)
            vext_sb = persist.tile([128, DT, 66], bf16)  # V blocks [t,hd] + ones col
            at_sb = persist.tile([128, 2, S], bf16)    # normalized attn out (j-major)

            # ---- PE warm-up: tiny matmuls during the DMA lead-in ----
            # keeps the HAM activity window busy so the first real matmuls run
            # at 2.4GHz instead of the cold 1.2GHz default. warm_src is
            # memset-initialized so the warm-up has no DMA dependency (input
            # DMA takes ~5us to deliver the first bytes).
            warm_src = persist.tile([64, 64], bf16)
            nc.vector.memset(warm_src, 1.0)
            ones_sb = persist.tile([128, 64], f32)
            nc.vector.memset(ones_sb[0:1, :], 1.0)
            warm_ps = qkv_ps.tile([64, 64], f32, tag="mm")
            for i in range(32):
                nc.tensor.matmul(
                    warm_ps, warm_src, warm_src,
                    start=(i == 0), stop=(i == 31))

            # ---- RoPE on a 512-col chunk ----
            # within each 64-row block: rows 0:32 even comps, 32:64 odd comps
            # roped = q * C + swap(q) * S  (C=[cos x4], S=[-sin,+sin]x2)
            def rope_into(dst, src, sc0, nrows, cc0):
                swp = rtmp_p.tile([128, 512], bf16, tag="swap")
                for b in range(nrows // 64):
                    nc.gpsimd.dma_start(out=swp[b * 64:b * 64 + 32, :], in_=src[b * 64 + 32:b * 64 + 64, sc0:sc0 + 512])
                    nc.gpsimd.dma_start(out=swp[b * 64 + 32:b * 64 + 64, :], in_=src[b * 64:b * 64 + 32, sc0:sc0 + 512])
                t1 = rtmp_p.tile([128, 512], bf16, tag="ropetmp")
                nc.vector.tensor_mul(t1[:nrows], src[:nrows, sc0:sc0 + 512], cos_sb[0:nrows, cc0:cc0 + 512])
                nc.vector.tensor_mul(swp[:nrows], swp[:nrows], sin_sb[0:nrows, cc0:cc0 + 512])
                nc.vector.tensor_add(dst, t1[:nrows], swp[:nrows])

            # ---- chunk projection: KV + rope + V transposes + Q + rope ----
            # emitted as a generator; each `yield` follows one PE instruction
            # so attention loops can pump it as PE filler.
            def proj_gen(c):
                # dt-half interleave (KV 0:8, Q0 0:8, KV 8:16, Q0 8:16, Q1)
                # so chunk-0 compute starts as soon as the first half of the
                # xt chunk and wq have landed; at most 2 PSUM slots live.
                c0, c1 = 512 * c, 512 * (c + 1)
                ps = qkv_ps.tile([128, 512], f32, tag="mm")
                ps2 = qkv_ps.tile([128, 512], f32, tag="mm")
                for dt in range(8):
                    nc.tensor.matmul(
                        ps, wkv_sb[:, dt, :], xt_sb[:, dt, c0:c1],
                        start=(dt == 0), stop=False)
                    yield
                for dt in range(8):
                    nc.tensor.matmul(
                        ps2, wq_sb[:, dt, 0:128], xt_sb[:, dt, c0:c1],
                        start=(dt == 0), stop=False)
                    yield
                for dt in range(8, DT):
                    nc.tensor.matmul(
                        ps, wkv_sb[:, dt, :], xt_sb[:, dt, c0:c1],
                        start=False, stop=(dt == DT - 1))
                    yield
                nc.vector.tensor_copy(kv2_sb[:, c0:c1], ps)
                rope_into(kv2_sb[0:64, c0:c1], kv2_sb, c0, 64, c0)
                nc.gpsimd.dma_start(out=ko_sb[64:128, c0:c1], in_=kv2_sb[0:64, c0:c1])
                for dt in range(8, DT):
                    nc.tensor.matmul(
                        ps2, wq_sb[:, dt, 0:128], xt_sb[:, dt, c0:c1],
                        start=False, stop=(dt == DT - 1))
                    yield
                qst = qstage_p.tile([128, 512], bf16, tag="qstage")
                nc.vector.tensor_copy(qst, ps2)
                rope_into(qtr_sb[:, 0, c0:c1], qst, 0, 128, c0)
                for tt in range(4 * c, 4 * c + 4):
                    vps = qkv_ps.tile([128, 64], bf16, tag="mm")
                    nc.tensor.transpose(vps, kv2_sb[64:128, tt * 128:(tt + 1) * 128], ident_sb[64:128, :])
                    nc.vector.tensor_copy(vext_sb[:, tt, 0:64], vps)
                    nc.vector.memset(vext_sb[:, tt, 64:65], 1.0)
                    yield
                ps3 = qkv_ps.tile([128, 512], f32, tag="mm")
                for dt in range(DT):
                    nc.tensor.matmul(
                        ps3, wq_sb[:, dt, 128:256], xt_sb[:, dt, c0:c1],
                        start=(dt == 0), stop=(dt == DT - 1))
                    yield
                qst2 = qstage_p.tile([128, 512], bf16, tag="qstage")
                nc.vector.tensor_copy(qst2, ps3)
                rope_into(qtr_sb[:, 1, c0:c1], qst2, 0, 128, c0)
                yield

            # ---- partial out-projection of superblock so (generator) ----
            # staging for the last superblock's output: batched into 4-dt
            # groups so the tail pays 4 DMA issues instead of 16 (each issue
            # is ~1us of queue time, which dominated the kernel tail).
            otail_sb = persist.tile([128, DT, 512], bf16)
            out_r = out_t.ap()[:, (SB - 1) * 512:SB * 512].rearrange(
                "(dt p) s -> p dt s", p=128)

            def outproj_gen(so):
                last = so == SB - 1
                for dt in range(DT):
                    po = qkv_ps.tile([128, 512], f32, tag="mm")
                    for jt in range(2):
                        nc.tensor.matmul(
                            po, wo_sb[:, jt, dt * 128:(dt + 1) * 128],
                            at_sb[:, jt, so * 512:(so + 1) * 512],
                            start=(jt == 0), stop=(jt == 1))
                    if last:
                        ost = otail_sb[:, dt, :]
                    else:
                        ost = ostage_p.tile([128, 512], bf16)
                    if last and dt % 2 == 0:
                        nc.scalar.activation(ost, po, mybir.ActivationFunctionType.Copy)
                    else:
                        nc.vector.tensor_copy(ost, po)
                    if last:
                        if dt % 4 == 3:
                            dtg = dt - 3
                            dq = nc.gpsimd if (dt // 4) % 2 == 1 else nc.sync
                            dq.dma_start(
                                out=out_r[:, dtg:dtg + 4, :],
                                in_=otail_sb[:, dtg:dtg + 4, :])
                    else:
                        nc.sync.dma_start(
                            out=out_t.ap()[dt * 128:(dt + 1) * 128, so * 512:(so + 1) * 512],
                            in_=ost)
                    yield

            def pump(itr, n):
                for _ in range(n):
                    try:
                        next(itr)
                    except StopIteration:
                        return

            # ---- main pipeline ----
            for _ in proj_gen(0):
                pass

            import itertools as _it
            for si in range(SB):
                nblk = 4 * (si + 1)
                c0, c1 = si * 512, (si + 1) * 512
                gens = []
                npump = 0
                if si < SB - 1:
                    gens.append(proj_gen(si + 1))
                    npump += 3 * DT + 4 + 1
                if si >= 1:
                    gens.append(outproj_gen(si - 1))
                    npump += DT
                filler = _it.chain(*gens)
                # finish the filler a few iterations early so no leftover
                # burst sits between this superblock and the next one's scores
                iters = max(4, 2 * nblk - 6)
                per_iter = -(-npump // iters) if npump else 0

                norm_b = []  # deferred phase-B closures
                for jt in range(2):  # head pair (2jt, 2jt+1)
                    ut0 = ut_ps.tile([65, 512], f32, tag="ut")
                    ut1 = ut_ps.tile([65, 512], f32, tag="ut")

                    # scores for t-block j, both heads of the pair, at PE row
                    # strips 0/64 (concurrent). Diagonal blocks only cover
                    # cols >= the block start.
                    def emit_scores(j, jt=jt, c0=c0, c1=c1, si=si):
                        jj = j - 4 * si
                        s0 = 128 * jj if jj > 0 else 0
                        st2 = st_ps.tile([128, 2, 512], f32, tag="st")
                        nc.tensor.matmul(
                            st2[:, 0, s0:512],
                            kv2_sb[0:64, j * 128:(j + 1) * 128],
                            qtr_sb[0:64, jt, c0 + s0:c1], start=True, stop=True)
                        nc.tensor.matmul(
                            st2[:, 1, s0:512],
                            ko_sb[64:128, j * 128:(j + 1) * 128],
                            qtr_sb[64:128, jt, c0 + s0:c1], start=True, stop=True)
                        return st2, jj, s0

                    cur = emit_scores(0)
                    for j in range(nblk):
                        # scores(j+1) go ahead of exp(j)/AV(j) in the PE
                        # stream so ScalarE is never waiting on a filler burst
                        nxt = emit_scores(j + 1) if j + 1 < nblk else None
                        st2, jj, s0 = cur
                        et2 = etp.tile([128, 2, 512], bf16, tag="et")
                        nc.scalar.activation(
                            et2[:, :, s0:512], st2[:, :, s0:512],
                            mybir.ActivationFunctionType.Exp)
                        if jj >= 0:
                            nc.vector.tensor_mul(
                                et2[:, 0, s0:s0 + 128], et2[:, 0, s0:s0 + 128], tri_sb)
                            nc.vector.tensor_mul(
                                et2[:, 1, s0:s0 + 128], et2[:, 1, s0:s0 + 128], tri_sb)
                        # trimmed accumulation is safe: j=0 is always a full-
                        # width start=True matmul, so every PSUM column is
                        # initialized before any partial-width accumulate.
                        nc.tensor.matmul(
                            ut0[:, s0:512], vext_sb[:, j, 0:65], et2[:, 0, s0:512],
                            start=(j == 0), stop=(j == nblk - 1),
                            skip_group_check=True)
                        nc.tensor.matmul(
                            ut1[:, s0:512], vext_sb[:, j, 0:65], et2[:, 1, s0:512],
                            start=(j == 0), stop=(j == nblk - 1),
                            skip_group_check=True)
                        pump(filler, per_iter)
                        cur = nxt
                    # normalize: at = ut[0:64] / ut[64]. Phase A (now): evacuate
                    # ut to SBUF (frees the PSUM slot) + reciprocal of the
                    # denominator row. Phase B (deferred past the next pair so
                    # the PE broadcast matmul never head-of-line-blocks the PE
                    # queue): broadcast 1/d across partitions via a ones-column
                    # matmul, then one multiply into at_sb.
                    for half, ut in ((0, ut0), (1, ut1)):
                        utsb = normp.tile([65, 512], f32, tag="utsb")
                        nc.vector.tensor_copy(utsb, ut)
                        den0 = normp.tile([1, 512], f32, tag="den0")
                        nc.gpsimd.dma_start(out=den0, in_=utsb[64:65, :])
                        rc = normp.tile([1, 512], f32, tag="recip")
                        nc.vector.reciprocal_approx_fast(rc, den0)

                        def phase_b(jt=jt, half=half, utsb=utsb, rc=rc):
                            bc = ut_ps.tile([64, 512], f32, tag="ut")
                            nc.tensor.matmul(
                                bc, ones_sb[0:1, :], rc,
                                start=True, stop=True)
                            if half == 0:
                                nc.vector.tensor_mul(
                                    at_sb[0:64, jt, c0:c1], utsb[0:64, :], bc)
                            else:
                                tmp64 = normp.tile([64, 512], bf16, tag="tmp64")
                                nc.vector.tensor_mul(tmp64, utsb[0:64, :], bc)
                                nc.gpsimd.dma_start(
                                    out=at_sb[64:128, jt, c0:c1], in_=tmp64)
                        norm_b.append(phase_b)
                for b in norm_b:
                    b()
                # drain any leftover filler before the next superblock
                for _ in filler:
                    pass

            for _ in outproj_gen(SB - 1):
                pass

    nc.compile()
    return nc


_SIGMA = np.concatenate([np.arange(0, HD, 2), np.arange(1, HD, 2)])


def _prep_inputs(x, freqs_cis, wq, wk, wv, wo):
    """Host-side shard + layout prep. Returns per-core in_maps."""
    x = np.asarray(x, np.float32).reshape(S, D)
    freqs_cis = np.asarray(freqs_cis, np.float32)
    wq = np.asarray(wq, np.float32)
    wk = np.asarray(wk, np.float32)
    wv = np.asarray(wv, np.float32)
    wo = np.asarray(wo, np.float32)

    xt = np.ascontiguousarray(x.T).astype(BF16)

    cosT = np.ascontiguousarray(freqs_cis[:, :, 0].T)  # [32, S]
    sinT = np.ascontiguousarray(freqs_cis[:, :, 1].T)
    cosb = np.ascontiguousarray(np.tile(cosT, (4, 1))).astype(BF16)
    sinb = np.ascontiguousarray(
        np.concatenate([-sinT, sinT, -sinT, sinT], 0)).astype(BF16)

    tloc = np.arange(128)[:, None]
    sloc = np.arange(128)[None, :]
    tri = (tloc <= sloc).astype(np.float32).astype(BF16)
    ident64 = np.eye(64, dtype=np.float32).astype(BF16)

    scale = 1.0 / np.sqrt(HD)
    in_maps = []
    for g in range(NC):
        wqg = wq[g * R * HD:(g + 1) * R * HD].reshape(R, HD, D)[:, _SIGMA, :].reshape(R * HD, D)
        wq_tg = np.ascontiguousarray(wqg.T).astype(BF16)
        wkg = wk[g * HD:(g + 1) * HD][_SIGMA] * scale
        wvg = wv[g * HD:(g + 1) * HD]
        wkv_tg = np.ascontiguousarray(np.concatenate([wkg, wvg], 0).T).astype(BF16)
        wo_tg = np.ascontiguousarray(wo[:, g * R * HD:(g + 1) * R * HD].T).astype(BF16)
        in_maps.append({
            "xt": xt,
            "wq_t": wq_tg,
            "wkv_t": wkv_tg,
            "wo_t": wo_tg,
            "cosb": cosb,
            "sinb": sinb,
            "tri": tri,
            "ident64": ident64,
        })
    return in_maps


_CACHED = {}


def _get_program():
    if "nc" not in _CACHED:
        _CACHED["nc"] = _build_program()
    return _CACHED["nc"]


def kernel(x, freqs_cis, wq, wk, wv, wo, _trace=False):
    from concourse.bass_utils import run_bass_kernel_spmd

    nc = _get_program()
    in_maps = _prep_inputs(x, freqs_cis, wq, wk, wv, wo)
    res = run_bass_kernel_spmd(nc, in_maps, core_ids=list(range(NC)), trace=_trace)
    acc = np.zeros((D, S), np.float64)
    for c in range(NC):
        acc += res.results[c]["out_t"].astype(np.float64)
    out = np.ascontiguousarray(acc.T, dtype=np.float32).reshape(1, S, D)
    if _trace:
        return out, res
    return out


# revision 29
# speedup vs baseline: 1.1737x; 1.1737x over previous
"""Tensor-parallel Llama GQA attention layer (B=1, S=2048, D=2048, H=32, KV=8)
for 8 Trainium2 NeuronCores.

Sharding: one KV group per core (kv head g + its 4 q heads). Each core computes
its heads' attention and a partial out-projection (contraction over its 256
head-dim columns of wo); the host sums the 8 bf16 partials (the TP all-reduce)
and transposes back to [1, S, D].

On-core layout is feature-major (transposed): xt=[D,S], QT=[j,S], KT/VT=[hd,S].
Scores are built per (head-pair, s-superblock of 512, t-block of 128) as
ST=[t,s] tiles; softmax is unnormalized exp with the denominator from a
ones-column appended to V, one normalization divide at the end.

Structure (vs the 243us baseline; measured ~231us on a nominal-clock device,
device thermal throttling adds up to ~20% run-to-run):
- input DMAs ordered by first use with xt chunk 0 split in dt-halves and the
  chunk-0 projection interleaved KV/Q so compute tracks DMA arrival; 32 tiny
  matmuls on a memset tile warm the PE HAM clock during the DMA lead-in
  (cold K=4/8 halves the PE clock).
- per-superblock software pipeline: attention(si) is emitted with scores
  lookahead-1 (scores j+1 ahead of exp/AV of j in the PE stream, so ScalarE
  exp runs back-to-back) and interleaved with the projection of chunk si+1
  and the out-projection of superblock si-1 as PE filler pumped between
  iterations; pumped projections hold one PSUM slot so the outproj filler
  always has the other.
- causal trimming at 128-col granularity: for diagonal t-blocks the scores
  matmul, exp, and AV matmul only cover columns >= the block start (safe:
  j=0 of every accumulation group is full-width start=True, so every PSUM
  column is initialized before any partial-width accumulate); the mask is a
  [128,128] lower-tri multiply per diagonal block.
- softmax normalization: denominator row moved to partition 0 (gpsimd DMA),
  reciprocal on DVE, then broadcast across partitions via a ones-column PE
  matmul (deferred past the next pair so it never head-of-line-blocks the PE
  queue) and one multiply. NOTE: the reciprocal/broadcast must run at
  partition 0 — running them at partition 64 NaN'd on hardware.
- output partials in bf16 (halves the 16.8MB output DMA); the last
  superblock's output DMAs alternate sync/gpsimd queues to halve the tail
  drain.
"""

import numpy as np
import ml_dtypes

S = 2048
D = 2048
H = 32
KV = 8
HD = 64
R = 4  # heads per kv group
NC = 8  # cores

BF16 = ml_dtypes.bfloat16


def _build_program():
    import concourse.mybir as mybir
    import concourse.tile as tile
    from concourse import bacc

    f32 = mybir.dt.float32
    bf16 = mybir.dt.bfloat16

    nc = bacc.Bacc("TRN2", debug=False, num_devices=NC)

    xt = nc.dram_tensor("xt", [D, S], bf16, kind="ExternalInput")
    wq_t = nc.dram_tensor("wq_t", [D, R * HD], bf16, kind="ExternalInput")
    wkv_t = nc.dram_tensor("wkv_t", [D, 2 * HD], bf16, kind="ExternalInput")
    wo_t = nc.dram_tensor("wo_t", [R * HD, D], bf16, kind="ExternalInput")
    cosb = nc.dram_tensor("cosb", [128, S], bf16, kind="ExternalInput")
    sinb = nc.dram_tensor("sinb", [128, S], bf16, kind="ExternalInput")
    tri = nc.dram_tensor("tri", [128, 128], bf16, kind="ExternalInput")
    ident64 = nc.dram_tensor("ident64", [64, 64], bf16, kind="ExternalInput")
    out_t = nc.dram_tensor("out_t", [D, S], bf16, kind="ExternalOutput")

    DT = D // 128  # 16 d tiles
    SB = S // 512  # 4 s superblocks

    with tile.TileContext(nc) as tc:
        with (
            tc.tile_pool(name="persist", bufs=1) as persist,
            tc.tile_pool(name="qstage", bufs=2) as qstage_p,
            tc.tile_pool(name="rtmp", bufs=2) as rtmp_p,
            tc.tile_pool(name="et", bufs=6) as etp,
            tc.tile_pool(name="norm", bufs=4) as normp,
            tc.tile_pool(name="ostage", bufs=3) as ostage_p,
            tc.tile_pool(name="qkv_ps", bufs=2, space="PSUM") as qkv_ps,
            tc.tile_pool(name="st_ps", bufs=2, space="PSUM") as st_ps,
            tc.tile_pool(name="ut_ps", bufs=2, space="PSUM") as ut_ps,
        ):
            # ---- persistent SBUF tensors + input DMA (ordered by first use) --
            xt_sb = persist.tile([128, DT, S], bf16)
            wq_sb = persist.tile([128, DT, R * HD], bf16)
            wkv_sb = persist.tile([128, DT, 2 * HD], bf16)
            wo_sb = persist.tile([128, 2, D], bf16)
            cos_sb = persist.tile([128, S], bf16)
            sin_sb = persist.tile([128, S], bf16)
            tri_sb = persist.tile([128, 128], bf16)
            ident_sb = persist.tile([128, 64], bf16)
            wkv_r = wkv_t.ap().rearrange("(dt p) j -> p dt j", p=128)
            wq_r = wq_t.ap().rearrange("(dt p) j -> p dt j", p=128)
            xt_r = xt.ap().rearrange("(dt p) s -> p dt s", p=128)
            # bulk activations/weights on the sync queue, ordered by first use;
            # small constants on the (otherwise idle) gpsimd queue so they
            # don't delay the xt stream.
            nc.sync.dma_start(out=wkv_sb, in_=wkv_r)
            nc.sync.dma_start(out=xt_sb[:, 0:4, 0:512], in_=xt_r[:, 0:4, 0:512])
            nc.sync.dma_start(out=xt_sb[:, 4:8, 0:512], in_=xt_r[:, 4:8, 0:512])
            nc.sync.dma_start(out=wq_sb, in_=wq_r)
            nc.sync.dma_start(out=xt_sb[:, 8:16, 0:512], in_=xt_r[:, 8:16, 0:512])
            nc.gpsimd.dma_start(out=ident_sb[64:128, :], in_=ident64.ap())
            nc.gpsimd.dma_start(out=cos_sb, in_=cosb.ap())
            nc.gpsimd.dma_start(out=sin_sb, in_=sinb.ap())
            nc.gpsimd.dma_start(out=tri_sb, in_=tri.ap())
            for sc in range(1, 4):
                nc.sync.dma_start(
                    out=xt_sb[:, :, sc * 512:(sc + 1) * 512],
                    in_=xt_r[:, :, sc * 512:(sc + 1) * 512])
            for jt in range(2):
                nc.sync.dma_start(out=wo_sb[:, jt, :], in_=wo_t.ap()[jt * 128:(jt + 1) * 128, :])

            qtr_sb = persist.tile([128, 2, S], bf16)   # roped Q, head-major
            kv2_sb = persist.tile([128, S], bf16)      # 0:64 roped K, 64:128 VT
            ko_sb = persist.tile([128, S], bf16)       # 64:128 roped K (odd heads)
            vext_sb = persist.tile([128, DT, 66], bf16)  # V blocks [t,hd] + ones col
            at_sb = persist.tile([128, 2, S], bf16)    # normalized attn out (j-major)

            # ---- PE warm-up: tiny matmuls during the DMA lead-in ----
            # keeps the HAM activity window busy so the first real matmuls run
            # at 2.4GHz instead of the cold 1.2GHz default. warm_src is
            # memset-initialized so the warm-up has no DMA dependency (input
            # DMA takes ~5us to deliver the first bytes).
            warm_src = persist.tile([64, 64], bf16)
            nc.vector.memset(warm_src, 1.0)
            ones_sb = persist.tile([128, 64], f32)
            nc.vector.memset(ones_sb[0:1, :], 1.0)
            warm_ps = qkv_ps.tile([64, 64], f32, tag="mm")
            for i in range(32):
                nc.tensor.matmul(
                    warm_ps, warm_src, warm_src,
                    start=(i == 0), stop=(i == 31))

            # ---- RoPE on a 512-col chunk ----
            # within each 64-row block: rows 0:32 even comps, 32:64 odd comps
            # roped = q * C + swap(q) * S  (C=[cos x4], S=[-sin,+sin]x2)
            def rope_into(dst, src, sc0, nrows, cc0):
                swp = rtmp_p.tile([128, 512], bf16, tag="swap")
                for b in range(nrows // 64):
                    nc.gpsimd.dma_start(out=swp[b * 64:b * 64 + 32, :], in_=src[b * 64 + 32:b * 64 + 64, sc0:sc0 + 512])
                    nc.gpsimd.dma_start(out=swp[b * 64 + 32:b * 64 + 64, :], in_=src[b * 64:b * 64 + 32, sc0:sc0 + 512])
                t1 = rtmp_p.tile([128, 512], bf16, tag="ropetmp")
                nc.vector.tensor_mul(t1[:nrows], src[:nrows, sc0:sc0 + 512], cos_sb[0:nrows, cc0:cc0 + 512])
                nc.vector.tensor_mul(swp[:nrows], swp[:nrows], sin_sb[0:nrows, cc0:cc0 + 512])
                nc.vector.tensor_add(dst, t1[:nrows], swp[:nrows])

            # ---- chunk projection: KV + rope + V transposes + Q + rope ----
            # emitted as a generator; each `yield` follows one PE instruction
            # so attention loops can pump it as PE filler.
            def proj_gen(c):
                # For chunk 0 (startup, nothing else on the PE) the KV and
                # first-Q matmuls interleave in dt-halves so compute tracks
                # the DMA arrival; for pumped chunks c>=1 the groups run
                # sequentially so only ONE qkv PSUM slot is held at a time,
                # leaving the second slot free for the outproj filler.
                c0, c1 = 512 * c, 512 * (c + 1)
                ps = qkv_ps.tile([128, 512], f32, tag="mm")
                if c == 0:
                    ps2 = qkv_ps.tile([128, 512], f32, tag="mm")
                    for dt in range(8):
                        nc.tensor.matmul(
                            ps, wkv_sb[:, dt, :], xt_sb[:, dt, c0:c1],
                            start=(dt == 0), stop=False)
                        yield
                    for dt in range(8):
                        nc.tensor.matmul(
                            ps2, wq_sb[:, dt, 0:128], xt_sb[:, dt, c0:c1],
                            start=(dt == 0), stop=False)
                        yield
                    for dt in range(8, DT):
                        nc.tensor.matmul(
                            ps, wkv_sb[:, dt, :], xt_sb[:, dt, c0:c1],
                            start=False, stop=(dt == DT - 1))
                        yield
                    nc.vector.tensor_copy(kv2_sb[:, c0:c1], ps)
                    rope_into(kv2_sb[0:64, c0:c1], kv2_sb, c0, 64, c0)
                    nc.gpsimd.dma_start(out=ko_sb[64:128, c0:c1], in_=kv2_sb[0:64, c0:c1])
                    for dt in range(8, DT):
                        nc.tensor.matmul(
                            ps2, wq_sb[:, dt, 0:128], xt_sb[:, dt, c0:c1],
                            start=False, stop=(dt == DT - 1))
                        yield
                else:
                    for dt in range(DT):
                        nc.tensor.matmul(
                            ps, wkv_sb[:, dt, :], xt_sb[:, dt, c0:c1],
                            start=(dt == 0), stop=(dt == DT - 1))
                        yield
                    nc.vector.tensor_copy(kv2_sb[:, c0:c1], ps)
                    rope_into(kv2_sb[0:64, c0:c1], kv2_sb, c0, 64, c0)
                    nc.gpsimd.dma_start(out=ko_sb[64:128, c0:c1], in_=kv2_sb[0:64, c0:c1])
                    ps2 = qkv_ps.tile([128, 512], f32, tag="mm")
                    for dt in range(DT):
                        nc.tensor.matmul(
                            ps2, wq_sb[:, dt, 0:128], xt_sb[:, dt, c0:c1],
                            start=(dt == 0), stop=(dt == DT - 1))
                        yield
                qst = qstage_p.tile([128, 512], bf16, tag="qstage")
                nc.vector.tensor_copy(qst, ps2)
                rope_into(qtr_sb[:, 0, c0:c1], qst, 0, 128, c0)
                for tt in range(4 * c, 4 * c + 4):
                    vps = qkv_ps.tile([128, 64], bf16, tag="mm")
                    nc.tensor.transpose(vps, kv2_sb[64:128, tt * 128:(tt + 1) * 128], ident_sb[64:128, :])
                    nc.vector.tensor_copy(vext_sb[:, tt, 0:64], vps)
                    nc.vector.memset(vext_sb[:, tt, 64:65], 1.0)
                    yield
                ps3 = qkv_ps.tile([128, 512], f32, tag="mm")
                for dt in range(DT):
                    nc.tensor.matmul(
                        ps3, wq_sb[:, dt, 128:256], xt_sb[:, dt, c0:c1],
                        start=(dt == 0), stop=(dt == DT - 1))
                    yield
                qst2 = qstage_p.tile([128, 512], bf16, tag="qstage")
                nc.vector.tensor_copy(qst2, ps3)
                rope_into(qtr_sb[:, 1, c0:c1], qst2, 0, 128, c0)
                yield

            # ---- partial out-projection of superblock so (generator) ----
            def outproj_gen(so):
                for dt in range(DT):
                    po = qkv_ps.tile([128, 512], f32, tag="mm")
                    for jt in range(2):
                        nc.tensor.matmul(
                            po, wo_sb[:, jt, dt * 128:(dt + 1) * 128],
                            at_sb[:, jt, so * 512:(so + 1) * 512],
                            start=(jt == 0), stop=(jt == 1))
                    ost = ostage_p.tile([128, 512], bf16)
                    if so == SB - 1 and dt % 2 == 0:
                        nc.scalar.activation(ost, po, mybir.ActivationFunctionType.Copy)
                    else:
                        nc.vector.tensor_copy(ost, po)
                    # split the last superblock's output DMAs across two
                    # queues so the final drain isn't serialized on one FIFO
                    dq = nc.gpsimd if (so == SB - 1 and dt % 2 == 1) else nc.sync
                    dq.dma_start(
                        out=out_t.ap()[dt * 128:(dt + 1) * 128, so * 512:(so + 1) * 512],
                        in_=ost)
                    yield

            def pump(itr, n):
                for _ in range(n):
                    try:
                        next(itr)
                    except StopIteration:
                        return

            # ---- main pipeline ----
            for _ in proj_gen(0):
                pass

            import itertools as _it
            for si in range(SB):
                nblk = 4 * (si + 1)
                c0, c1 = si * 512, (si + 1) * 512
                gens = []
                npump = 0
                if si < SB - 1:
                    gens.append(proj_gen(si + 1))
                    npump += 3 * DT + 4 + 1
                if si >= 1:
                    gens.append(outproj_gen(si - 1))
                    npump += DT
                filler = _it.chain(*gens)
                iters = 2 * nblk
                per_iter = -(-npump // iters) if npump else 0

                norm_b = []  # deferred phase-B closures
                for jt in range(2):  # head pair (2jt, 2jt+1)
                    ut0 = ut_ps.tile([65, 512], f32, tag="ut")
                    ut1 = ut_ps.tile([65, 512], f32, tag="ut")

                    # scores for t-block j, both heads of the pair, at PE row
                    # strips 0/64 (concurrent). Diagonal blocks only cover
                    # cols >= the block start.
                    def emit_scores(j, jt=jt, c0=c0, c1=c1, si=si):
                        jj = j - 4 * si
                        s0 = 128 * jj if jj > 0 else 0
                        st2 = st_ps.tile([128, 2, 512], f32, tag="st")
                        nc.tensor.matmul(
                            st2[:, 0, s0:512],
                            kv2_sb[0:64, j * 128:(j + 1) * 128],
                            qtr_sb[0:64, jt, c0 + s0:c1], start=True, stop=True)
                        nc.tensor.matmul(
                            st2[:, 1, s0:512],
                            ko_sb[64:128, j * 128:(j + 1) * 128],
                            qtr_sb[64:128, jt, c0 + s0:c1], start=True, stop=True)
                        return st2, jj, s0

                    cur = emit_scores(0)
                    for j in range(nblk):
                        # scores(j+1) go ahead of exp(j)/AV(j) in the PE
                        # stream so ScalarE is never waiting on a filler burst
                        nxt = emit_scores(j + 1) if j + 1 < nblk else None
                        st2, jj, s0 = cur
                        et2 = etp.tile([128, 2, 512], bf16, tag="et")
                        nc.scalar.activation(
                            et2[:, :, s0:512], st2[:, :, s0:512],
                            mybir.ActivationFunctionType.Exp)
                        if jj >= 0:
                            nc.vector.tensor_mul(
                                et2[:, 0, s0:s0 + 128], et2[:, 0, s0:s0 + 128], tri_sb)
                            nc.vector.tensor_mul(
                                et2[:, 1, s0:s0 + 128], et2[:, 1, s0:s0 + 128], tri_sb)
                        # trimmed accumulation is safe: j=0 is always a full-
                        # width start=True matmul, so every PSUM column is
                        # initialized before any partial-width accumulate.
                        nc.tensor.matmul(
                            ut0[:, s0:512], vext_sb[:, j, 0:65], et2[:, 0, s0:512],
                            start=(j == 0), stop=(j == nblk - 1),
                            skip_group_check=True)
                        nc.tensor.matmul(
                            ut1[:, s0:512], vext_sb[:, j, 0:65], et2[:, 1, s0:512],
                            start=(j == 0), stop=(j == nblk - 1),
                            skip_group_check=True)
                        pump(filler, per_iter)
                        cur = nxt
                    # normalize: at = ut[0:64] / ut[64]. Phase A (now): evacuate
                    # ut to SBUF (frees the PSUM slot) + reciprocal of the
                    # denominator row. Phase B (deferred past the next pair so
                    # the PE broadcast matmul never head-of-line-blocks the PE
                    # queue): broadcast 1/d across partitions via a ones-column
                    # matmul, then one multiply into at_sb.
                    for half, ut in ((0, ut0), (1, ut1)):
                        utsb = normp.tile([65, 512], f32, tag="utsb")
                        nc.vector.tensor_copy(utsb, ut)
                        den0 = normp.tile([1, 512], f32, tag="den0")
                        nc.gpsimd.dma_start(out=den0, in_=utsb[64:65, :])
                        rc = normp.tile([1, 512], f32, tag="recip")
                        nc.vector.reciprocal_approx_fast(rc, den0)

                        def phase_b(jt=jt, half=half, utsb=utsb, rc=rc):
                            bc = ut_ps.tile([64, 512], f32, tag="ut")
                            nc.tensor.matmul(
                                bc, ones_sb[0:1, :], rc,
                                start=True, stop=True)
                            if half == 0:
                                nc.vector.tensor_mul(
                                    at_sb[0:64, jt, c0:c1], utsb[0:64, :], bc)
                            else:
                                tmp64 = normp.tile([64, 512], bf16, tag="tmp64")
                                nc.vector.tensor_mul(tmp64, utsb[0:64, :], bc)
                                nc.gpsimd.dma_start(
                                    out=at_sb[64:128, jt, c0:c1], in_=tmp64)
                        norm_b.append(phase_b)
                for b in norm_b:
                    b()
                # drain any leftover filler before the next superblock
                for _ in filler:
                    pass

            for _ in outproj_gen(SB - 1):
                pass

    nc.compile()
    return nc


_SIGMA = np.concatenate([np.arange(0, HD, 2), np.arange(1, HD, 2)])


def _prep_inputs(x, freqs_cis, wq, wk, wv, wo):
    """Host-side shard + layout prep. Returns per-core in_maps."""
    x = np.asarray(x, np.float32).reshape(S, D)
    freqs_cis = np.asarray(freqs_cis, np.float32)
    wq = np.asarray(wq, np.float32)
    wk = np.asarray(wk, np.float32)
    wv = np.asarray(wv, np.float32)
    wo = np.asarray(wo, np.float32)

    xt = np.ascontiguousarray(x.T).astype(BF16)

    cosT = np.ascontiguousarray(freqs_cis[:, :, 0].T)  # [32, S]
    sinT = np.ascontiguousarray(freqs_cis[:, :, 1].T)
    cosb = np.ascontiguousarray(np.tile(cosT, (4, 1))).astype(BF16)
    sinb = np.ascontiguousarray(
        np.concatenate([-sinT, sinT, -sinT, sinT], 0)).astype(BF16)

    tloc = np.arange(128)[:, None]
    sloc = np.arange(128)[None, :]
    tri = (tloc <= sloc).astype(np.float32).astype(BF16)
    ident64 = np.eye(64, dtype=np.float32).astype(BF16)

    scale = 1.0 / np.sqrt(HD)
    in_maps = []
    for g in range(NC):
        wqg = wq[g * R * HD:(g + 1) * R * HD].reshape(R, HD, D)[:, _SIGMA, :].reshape(R * HD, D)
        wq_tg = np.ascontiguousarray(wqg.T).astype(BF16)
        wkg = wk[g * HD:(g + 1) * HD][_SIGMA] * scale
        wvg = wv[g * HD:(g + 1) * HD]
        wkv_tg = np.ascontiguousarray(np.concatenate([wkg, wvg], 0).T).astype(BF16)
        wo_tg = np.ascontiguousarray(wo[:, g * R * HD:(g + 1) * R * HD].T).astype(BF16)
        in_maps.append({
            "xt": xt,
            "wq_t": wq_tg,
            "wkv_t": wkv_tg,
            "wo_t": wo_tg,
            "cosb": cosb,
            "sinb": sinb,
            "tri": tri,
            "ident64": ident64,
        })
    return in_maps


_CACHED = {}


def _get_program():
    if "nc" not in _CACHED:
        _CACHED["nc"] = _build_program()
    return _CACHED["nc"]


def kernel(x, freqs_cis, wq, wk, wv, wo, _trace=False):
    from concourse.bass_utils import run_bass_kernel_spmd

    nc = _get_program()
    in_maps = _prep_inputs(x, freqs_cis, wq, wk, wv, wo)
    res = run_bass_kernel_spmd(nc, in_maps, core_ids=list(range(NC)), trace=_trace)
    acc = np.zeros((D, S), np.float64)
    for c in range(NC):
        acc += res.results[c]["out_t"].astype(np.float64)
    out = np.ascontiguousarray(acc.T, dtype=np.float32).reshape(1, S, D)
    if _trace:
        return out, res
    return out


# revision 31
# speedup vs baseline: 1.2170x; 1.0369x over previous
"""Tensor-parallel Llama GQA attention layer (B=1, S=2048, D=2048, H=32, KV=8)
for 8 Trainium2 NeuronCores.

Sharding: one KV group per core (kv head g + its 4 q heads). Each core computes
its heads' attention and a partial out-projection (contraction over its 256
head-dim columns of wo); the host sums the 8 bf16 partials (the TP all-reduce)
and transposes back to [1, S, D].

On-core layout is feature-major (transposed): xt=[D,S], QT=[j,S], KT/VT=[hd,S].
Scores are built per (head-pair, s-superblock of 512, t-block of 128) as
ST=[t,s] tiles; softmax is unnormalized exp with the denominator from a
ones-column appended to V, one normalization divide at the end.

Structure (vs the 243us baseline; measured ~231us on a nominal-clock device,
device thermal throttling adds up to ~20% run-to-run):
- input DMAs ordered by first use with xt chunk 0 split in dt-halves and the
  chunk-0 projection interleaved KV/Q so compute tracks DMA arrival; 32 tiny
  matmuls on a memset tile warm the PE HAM clock during the DMA lead-in
  (cold K=4/8 halves the PE clock).
- per-superblock software pipeline: attention(si) is emitted with scores
  lookahead-1 (scores j+1 ahead of exp/AV of j in the PE stream, so ScalarE
  exp runs back-to-back) and interleaved with the projection of chunk si+1
  and the out-projection of superblock si-1 as PE filler pumped between
  iterations; pumped projections hold one PSUM slot so the outproj filler
  always has the other.
- causal trimming at 128-col granularity: for diagonal t-blocks the scores
  matmul, exp, and AV matmul only cover columns >= the block start (safe:
  j=0 of every accumulation group is full-width start=True, so every PSUM
  column is initialized before any partial-width accumulate); the mask is a
  [128,128] lower-tri multiply per diagonal block.
- softmax normalization: denominator row moved to partition 0 (gpsimd DMA),
  reciprocal on DVE, then broadcast across partitions via a ones-column PE
  matmul (deferred past the next pair so it never head-of-line-blocks the PE
  queue) and one multiply. NOTE: the reciprocal/broadcast must run at
  partition 0 — running them at partition 64 NaN'd on hardware.
- output partials in bf16 (halves the 16.8MB output DMA); the last
  superblock's output DMAs alternate sync/gpsimd queues to halve the tail
  drain.
"""

import numpy as np
import ml_dtypes

S = 2048
D = 2048
H = 32
KV = 8
HD = 64
R = 4  # heads per kv group
NC = 8  # cores

BF16 = ml_dtypes.bfloat16


def _build_program():
    import concourse.mybir as mybir
    import concourse.tile as tile
    from concourse import bacc

    f32 = mybir.dt.float32
    bf16 = mybir.dt.bfloat16

    nc = bacc.Bacc("TRN2", debug=False, num_devices=NC)

    xt = nc.dram_tensor("xt", [D, S], bf16, kind="ExternalInput")
    wq_t = nc.dram_tensor("wq_t", [D, R * HD], bf16, kind="ExternalInput")
    wkv_t = nc.dram_tensor("wkv_t", [D, 2 * HD], bf16, kind="ExternalInput")
    wo_t = nc.dram_tensor("wo_t", [R * HD, D], bf16, kind="ExternalInput")
    cosb = nc.dram_tensor("cosb", [128, S], bf16, kind="ExternalInput")
    sinb = nc.dram_tensor("sinb", [128, S], bf16, kind="ExternalInput")
    tri = nc.dram_tensor("tri", [128, 128], bf16, kind="ExternalInput")
    ident64 = nc.dram_tensor("ident64", [64, 64], bf16, kind="ExternalInput")
    out_t = nc.dram_tensor("out_t", [D, S], bf16, kind="ExternalOutput")

    DT = D // 128  # 16 d tiles
    SB = S // 512  # 4 s superblocks

    with tile.TileContext(nc) as tc:
        with (
            tc.tile_pool(name="persist", bufs=1) as persist,
            tc.tile_pool(name="qstage", bufs=2) as qstage_p,
            tc.tile_pool(name="rtmp", bufs=2) as rtmp_p,
            tc.tile_pool(name="et", bufs=6) as etp,
            tc.tile_pool(name="norm", bufs=4) as normp,
            tc.tile_pool(name="ostage", bufs=3) as ostage_p,
            tc.tile_pool(name="qkv_ps", bufs=2, space="PSUM") as qkv_ps,
            tc.tile_pool(name="st_ps", bufs=2, space="PSUM") as st_ps,
            tc.tile_pool(name="ut_ps", bufs=2, space="PSUM") as ut_ps,
        ):
            # ---- persistent SBUF tensors + input DMA (ordered by first use) --
            xt_sb = persist.tile([128, DT, S], bf16)
            wq_sb = persist.tile([128, DT, R * HD], bf16)
            wkv_sb = persist.tile([128, DT, 2 * HD], bf16)
            wo_sb = persist.tile([128, 2, D], bf16)
            cos_sb = persist.tile([128, S], bf16)
            sin_sb = persist.tile([128, S], bf16)
            tri_sb = persist.tile([128, 128], bf16)
            ident_sb = persist.tile([128, 64], bf16)
            wkv_r = wkv_t.ap().rearrange("(dt p) j -> p dt j", p=128)
            wq_r = wq_t.ap().rearrange("(dt p) j -> p dt j", p=128)
            xt_r = xt.ap().rearrange("(dt p) s -> p dt s", p=128)
            # bulk activations/weights on the sync queue, ordered by first use;
            # small constants on the (otherwise idle) gpsimd queue so they
            # don't delay the xt stream.
            nc.sync.dma_start(out=wkv_sb, in_=wkv_r)
            nc.sync.dma_start(out=xt_sb[:, 0:4, 0:512], in_=xt_r[:, 0:4, 0:512])
            nc.sync.dma_start(out=xt_sb[:, 4:8, 0:512], in_=xt_r[:, 4:8, 0:512])
            nc.sync.dma_start(out=wq_sb, in_=wq_r)
            nc.sync.dma_start(out=xt_sb[:, 8:16, 0:512], in_=xt_r[:, 8:16, 0:512])
            nc.gpsimd.dma_start(out=ident_sb[64:128, :], in_=ident64.ap())
            nc.gpsimd.dma_start(out=cos_sb, in_=cosb.ap())
            nc.gpsimd.dma_start(out=sin_sb, in_=sinb.ap())
            nc.gpsimd.dma_start(out=tri_sb, in_=tri.ap())
            for sc in range(1, 4):
                nc.sync.dma_start(
                    out=xt_sb[:, :, sc * 512:(sc + 1) * 512],
                    in_=xt_r[:, :, sc * 512:(sc + 1) * 512])
            for jt in range(2):
                nc.sync.dma_start(out=wo_sb[:, jt, :], in_=wo_t.ap()[jt * 128:(jt + 1) * 128, :])

            qtr_sb = persist.tile([128, 2, S], bf16)   # roped Q, head-major
            kv2_sb = persist.tile([128, S], bf16)      # 0:64 roped K, 64:128 VT
            ko_sb = persist.tile([128, S], bf16)       # 64:128 roped K (odd heads)
            vext_sb = persist.tile([128, DT, 66], bf16)  # V blocks [t,hd] + ones col
            at_sb = persist.tile([128, 2, S], bf16)    # normalized attn out (j-major)

            # ---- PE warm-up: tiny matmuls during the DMA lead-in ----
            # keeps the HAM activity window busy so the first real matmuls run
            # at 2.4GHz instead of the cold 1.2GHz default. warm_src is
            # memset-initialized so the warm-up has no DMA dependency (input
            # DMA takes ~5us to deliver the first bytes).
            warm_src = persist.tile([64, 64], bf16)
            nc.vector.memset(warm_src, 1.0)
            ones_sb = persist.tile([128, 64], f32)
            nc.vector.memset(ones_sb[0:1, :], 1.0)
            warm_ps = qkv_ps.tile([64, 64], f32, tag="mm")
            for i in range(32):
                nc.tensor.matmul(
                    warm_ps, warm_src, warm_src,
                    start=(i == 0), stop=(i == 31))

            # ---- RoPE on a 512-col chunk ----
            # within each 64-row block: rows 0:32 even comps, 32:64 odd comps
            # roped = q * C + swap(q) * S  (C=[cos x4], S=[-sin,+sin]x2)
            def rope_into(dst, src, sc0, nrows, cc0):
                swp = rtmp_p.tile([128, 512], bf16, tag="swap")
                for b in range(nrows // 64):
                    nc.gpsimd.dma_start(out=swp[b * 64:b * 64 + 32, :], in_=src[b * 64 + 32:b * 64 + 64, sc0:sc0 + 512])
                    nc.gpsimd.dma_start(out=swp[b * 64 + 32:b * 64 + 64, :], in_=src[b * 64:b * 64 + 32, sc0:sc0 + 512])
                t1 = rtmp_p.tile([128, 512], bf16, tag="ropetmp")
                nc.vector.tensor_mul(t1[:nrows], src[:nrows, sc0:sc0 + 512], cos_sb[0:nrows, cc0:cc0 + 512])
                nc.vector.tensor_mul(swp[:nrows], swp[:nrows], sin_sb[0:nrows, cc0:cc0 + 512])
                nc.vector.tensor_add(dst, t1[:nrows], swp[:nrows])

            # ---- chunk projection: KV + rope + V transposes + Q + rope ----
            # emitted as a generator; each `yield` follows one PE instruction
            # so attention loops can pump it as PE filler.
            def proj_gen(c):
                # For chunk 0 (startup, nothing else on the PE) the KV and
                # first-Q matmuls interleave in dt-halves so compute tracks
                # the DMA arrival; for pumped chunks c>=1 the groups run
                # sequentially so only ONE qkv PSUM slot is held at a time,
                # leaving the second slot free for the outproj filler.
                c0, c1 = 512 * c, 512 * (c + 1)
                ps = qkv_ps.tile([128, 512], f32, tag="mm")
                if c == 0:
                    ps2 = qkv_ps.tile([128, 512], f32, tag="mm")
                    for dt in range(8):
                        nc.tensor.matmul(
                            ps, wkv_sb[:, dt, :], xt_sb[:, dt, c0:c1],
                            start=(dt == 0), stop=False)
                        yield
                    for dt in range(8):
                        nc.tensor.matmul(
                            ps2, wq_sb[:, dt, 0:128], xt_sb[:, dt, c0:c1],
                            start=(dt == 0), stop=False)
                        yield
                    for dt in range(8, DT):
                        nc.tensor.matmul(
                            ps, wkv_sb[:, dt, :], xt_sb[:, dt, c0:c1],
                            start=False, stop=(dt == DT - 1))
                        yield
                    nc.vector.tensor_copy(kv2_sb[:, c0:c1], ps)
                    rope_into(kv2_sb[0:64, c0:c1], kv2_sb, c0, 64, c0)
                    nc.gpsimd.dma_start(out=ko_sb[64:128, c0:c1], in_=kv2_sb[0:64, c0:c1])
                    for dt in range(8, DT):
                        nc.tensor.matmul(
                            ps2, wq_sb[:, dt, 0:128], xt_sb[:, dt, c0:c1],
                            start=False, stop=(dt == DT - 1))
                        yield
                else:
                    for dt in range(DT):
                        nc.tensor.matmul(
                            ps, wkv_sb[:, dt, :], xt_sb[:, dt, c0:c1],
                            start=(dt == 0), stop=(dt == DT - 1))
                        yield
                    nc.vector.tensor_copy(kv2_sb[:, c0:c1], ps)
                    rope_into(kv2_sb[0:64, c0:c1], kv2_sb, c0, 64, c0)
                    nc.gpsimd.dma_start(out=ko_sb[64:128, c0:c1], in_=kv2_sb[0:64, c0:c1])
                    ps2 = qkv_ps.tile([128, 512], f32, tag="mm")
                    for dt in range(DT):
                        nc.tensor.matmul(
                            ps2, wq_sb[:, dt, 0:128], xt_sb[:, dt, c0:c1],
                            start=(dt == 0), stop=(dt == DT - 1))
                        yield
                qst = qstage_p.tile([128, 512], bf16, tag="qstage")
                nc.vector.tensor_copy(qst, ps2)
                rope_into(qtr_sb[:, 0, c0:c1], qst, 0, 128, c0)
                for tt in range(4 * c, 4 * c + 4):
                    vps = qkv_ps.tile([128, 64], bf16, tag="mm")
                    nc.tensor.transpose(vps, kv2_sb[64:128, tt * 128:(tt + 1) * 128], ident_sb[64:128, :])
                    nc.vector.tensor_copy(vext_sb[:, tt, 0:64], vps)
                    nc.vector.memset(vext_sb[:, tt, 64:65], 1.0)
                    yield
                ps3 = qkv_ps.tile([128, 512], f32, tag="mm")
                for dt in range(DT):
                    nc.tensor.matmul(
                        ps3, wq_sb[:, dt, 128:256], xt_sb[:, dt, c0:c1],
                        start=(dt == 0), stop=(dt == DT - 1))
                    yield
                qst2 = qstage_p.tile([128, 512], bf16, tag="qstage")
                nc.vector.tensor_copy(qst2, ps3)
                rope_into(qtr_sb[:, 1, c0:c1], qst2, 0, 128, c0)
                yield

            # ---- partial out-projection of superblock so (generator) ----
            def outproj_gen(so):
                for dt in range(DT):
                    po = qkv_ps.tile([128, 512], f32, tag="mm")
                    for jt in range(2):
                        nc.tensor.matmul(
                            po, wo_sb[:, jt, dt * 128:(dt + 1) * 128],
                            at_sb[:, jt, so * 512:(so + 1) * 512],
                            start=(jt == 0), stop=(jt == 1))
                    ost = ostage_p.tile([128, 512], bf16)
                    if so == SB - 1 and dt % 2 == 0:
                        nc.scalar.activation(ost, po, mybir.ActivationFunctionType.Copy)
                    else:
                        nc.vector.tensor_copy(ost, po)
                    # split the last superblock's output DMAs across two
                    # queues so the final drain isn't serialized on one FIFO
                    dq = nc.gpsimd if (so == SB - 1 and dt % 2 == 1) else nc.sync
                    dq.dma_start(
                        out=out_t.ap()[dt * 128:(dt + 1) * 128, so * 512:(so + 1) * 512],
                        in_=ost)
                    yield

            def pump(itr, n):
                for _ in range(n):
                    try:
                        next(itr)
                    except StopIteration:
                        return

            # ---- main pipeline ----
            for _ in proj_gen(0):
                pass

            import itertools as _it
            for si in range(SB):
                nblk = 4 * (si + 1)
                c0, c1 = si * 512, (si + 1) * 512
                gens = []
                npump = 0
                if si < SB - 1:
                    gens.append(proj_gen(si + 1))
                    npump += 3 * DT + 4 + 1
                if si >= 1:
                    gens.append(outproj_gen(si - 1))
                    npump += DT
                filler = _it.chain(*gens)
                iters = 2 * nblk
                per_iter = -(-npump // iters) if npump else 0

                for jt in range(2):  # head pair (2jt, 2jt+1)
                    ut0 = ut_ps.tile([65, 512], f32, tag="ut")
                    ut1 = ut_ps.tile([65, 512], f32, tag="ut")

                    # scores for t-block j, both heads of the pair, at PE row
                    # strips 0/64 (concurrent). Diagonal blocks only cover
                    # cols >= the block start.
                    def emit_scores(j, jt=jt, c0=c0, c1=c1, si=si):
                        jj = j - 4 * si
                        s0 = 128 * jj if jj > 0 else 0
                        st2 = st_ps.tile([128, 2, 512], f32, tag="st")
                        nc.tensor.matmul(
                            st2[:, 0, s0:512],
                            kv2_sb[0:64, j * 128:(j + 1) * 128],
                            qtr_sb[0:64, jt, c0 + s0:c1], start=True, stop=True)
                        nc.tensor.matmul(
                            st2[:, 1, s0:512],
                            ko_sb[64:128, j * 128:(j + 1) * 128],
                            qtr_sb[64:128, jt, c0 + s0:c1], start=True, stop=True)
                        return st2, jj, s0

                    cur = emit_scores(0)
                    for j in range(nblk):
                        # scores(j+1) go ahead of exp(j)/AV(j) in the PE
                        # stream so ScalarE is never waiting on a filler burst
                        nxt = emit_scores(j + 1) if j + 1 < nblk else None
                        st2, jj, s0 = cur
                        et2 = etp.tile([128, 2, 512], bf16, tag="et")
                        nc.scalar.activation(
                            et2[:, :, s0:512], st2[:, :, s0:512],
                            mybir.ActivationFunctionType.Exp)
                        if jj >= 0:
                            nc.vector.tensor_mul(
                                et2[:, 0, s0:s0 + 128], et2[:, 0, s0:s0 + 128], tri_sb)
                            nc.vector.tensor_mul(
                                et2[:, 1, s0:s0 + 128], et2[:, 1, s0:s0 + 128], tri_sb)
                        # trimmed accumulation is safe: j=0 is always a full-
                        # width start=True matmul, so every PSUM column is
                        # initialized before any partial-width accumulate.
                        nc.tensor.matmul(
                            ut0[:, s0:512], vext_sb[:, j, 0:65], et2[:, 0, s0:512],
                            start=(j == 0), stop=(j == nblk - 1),
                            skip_group_check=True)
                        nc.tensor.matmul(
                            ut1[:, s0:512], vext_sb[:, j, 0:65], et2[:, 1, s0:512],
                            start=(j == 0), stop=(j == nblk - 1),
                            skip_group_check=True)
                        pump(filler, per_iter)
                        cur = nxt
                    # normalize: at = ut[0:64] / ut[64]. Evacuate the whole ut
                    # first (frees the PSUM slot immediately); the rest of the
                    # chain (denominator row to partition 0, reciprocal,
                    # gpsimd partition-broadcast, one multiply) runs off the
                    # critical path on DVE/gpsimd.
                    for half, ut in ((0, ut0), (1, ut1)):
                        utsb = normp.tile([65, 512], f32, tag="utsb")
                        nc.vector.tensor_copy(utsb, ut)
                        den0 = normp.tile([1, 512], f32, tag="den0")
                        nc.gpsimd.dma_start(out=den0, in_=utsb[64:65, :])
                        rc = normp.tile([1, 512], f32, tag="recip")
                        nc.vector.reciprocal_approx_fast(rc, den0)
                        bc = normp.tile([64, 512], f32, tag="bcast")
                        nc.gpsimd.partition_broadcast(bc, rc)
                        if half == 0:
                            nc.vector.tensor_mul(
                                at_sb[0:64, jt, c0:c1], utsb[0:64, :], bc)
                        else:
                            tmp64 = normp.tile([64, 512], bf16, tag="tmp64")
                            nc.vector.tensor_mul(tmp64, utsb[0:64, :], bc)
                            nc.gpsimd.dma_start(
                                out=at_sb[64:128, jt, c0:c1], in_=tmp64)
                # drain any leftover filler before the next superblock
                for _ in filler:
                    pass

            for _ in outproj_gen(SB - 1):
                pass

    nc.compile()
    return nc


_SIGMA = np.concatenate([np.arange(0, HD, 2), np.arange(1, HD, 2)])


def _prep_inputs(x, freqs_cis, wq, wk, wv, wo):
    """Host-side shard + layout prep. Returns per-core in_maps."""
    x = np.asarray(x, np.float32).reshape(S, D)
    freqs_cis = np.asarray(freqs_cis, np.float32)
    wq = np.asarray(wq, np.float32)
    wk = np.asarray(wk, np.float32)
    wv = np.asarray(wv, np.float32)
    wo = np.asarray(wo, np.float32)

    xt = np.ascontiguousarray(x.T).astype(BF16)

    cosT = np.ascontiguousarray(freqs_cis[:, :, 0].T)  # [32, S]
    sinT = np.ascontiguousarray(freqs_cis[:, :, 1].T)
    cosb = np.ascontiguousarray(np.tile(cosT, (4, 1))).astype(BF16)
    sinb = np.ascontiguousarray(
        np.concatenate([-sinT, sinT, -sinT, sinT], 0)).astype(BF16)

    tloc = np.arange(128)[:, None]
    sloc = np.arange(128)[None, :]
    tri = (tloc <= sloc).astype(np.float32).astype(BF16)
    ident64 = np.eye(64, dtype=np.float32).astype(BF16)

    scale = 1.0 / np.sqrt(HD)
    in_maps = []
    for g in range(NC):
        wqg = wq[g * R * HD:(g + 1) * R * HD].reshape(R, HD, D)[:, _SIGMA, :].reshape(R * HD, D)
        wq_tg = np.ascontiguousarray(wqg.T).astype(BF16)
        wkg = wk[g * HD:(g + 1) * HD][_SIGMA] * scale
        wvg = wv[g * HD:(g + 1) * HD]
        wkv_tg = np.ascontiguousarray(np.concatenate([wkg, wvg], 0).T).astype(BF16)
        wo_tg = np.ascontiguousarray(wo[:, g * R * HD:(g + 1) * R * HD].T).astype(BF16)
        in_maps.append({
            "xt": xt,
            "wq_t": wq_tg,
            "wkv_t": wkv_tg,
            "wo_t": wo_tg,
            "cosb": cosb,
            "sinb": sinb,
            "tri": tri,
            "ident64": ident64,
        })
    return in_maps


_CACHED = {}


def _get_program():
    if "nc" not in _CACHED:
        _CACHED["nc"] = _build_program()
    return _CACHED["nc"]


def kernel(x, freqs_cis, wq, wk, wv, wo, _trace=False):
    from concourse.bass_utils import run_bass_kernel_spmd

    nc = _get_program()
    in_maps = _prep_inputs(x, freqs_cis, wq, wk, wv, wo)
    res = run_bass_kernel_spmd(nc, in_maps, core_ids=list(range(NC)), trace=_trace)
    acc = np.zeros((D, S), np.float64)
    for c in range(NC):
        acc += res.results[c]["out_t"].astype(np.float64)
    out = np.ascontiguousarray(acc.T, dtype=np.float32).reshape(1, S, D)
    if _trace:
        return out, res
    return out


# revision 34
# speedup vs baseline: 1.2212x; 1.0035x over previous
"""Tensor-parallel Llama GQA attention layer (B=1, S=2048, D=2048, H=32, KV=8)
for 8 Trainium2 NeuronCores.

Sharding: one KV group per core (kv head g + its 4 q heads). Each core computes
its heads' attention and a partial out-projection (contraction over its 256
head-dim columns of wo); the host sums the 8 bf16 partials (the TP all-reduce)
and transposes back to [1, S, D].

On-core layout is feature-major (transposed): xt=[D,S], QT=[j,S], KT/VT=[hd,S].
Scores are built per (head-pair, s-superblock of 512, t-block of 128) as
ST=[t,s] tiles; softmax is unnormalized exp with the denominator from a
ones-column appended to V, one normalization divide at the end.

Structure (vs the 243us baseline; measured ~223us on a nominal-clock device,
device thermal throttling adds up to ~20% run-to-run):
- input DMAs ordered by first use with xt chunk 0 split in dt-halves and the
  chunk-0 projection interleaved KV/Q so compute tracks DMA arrival; 32 tiny
  matmuls on a memset tile warm the PE HAM clock during the DMA lead-in
  (cold K=4/8 halves the PE clock).
- per-superblock software pipeline: attention(si) is emitted with scores
  lookahead-1 (scores j+1 ahead of exp/AV of j in the PE stream, so ScalarE
  exp runs back-to-back) and interleaved with the projection of chunk si+1
  and the out-projection of superblock si-1 as PE filler pumped between
  iterations; pumped projections hold one PSUM slot so the outproj filler
  always has the other.
- causal trimming at 128-col granularity: for diagonal t-blocks the scores
  matmul, exp, and AV matmul only cover columns >= the block start (safe:
  j=0 of every accumulation group is full-width start=True, so every PSUM
  column is initialized before any partial-width accumulate); the mask is a
  [128,128] lower-tri multiply per diagonal block.
- softmax normalization: whole-ut evacuation (frees the PSUM slot at once),
  denominator row moved to partition 0 (gpsimd DMA), reciprocal on DVE,
  gpsimd partition_broadcast, one multiply. NOTE: the reciprocal/broadcast
  chain must run at partition 0 — running it at partition 64 NaN'd on
  hardware (a PE ones-matmul broadcast variant was also ~8us slower).
- output partials in bf16 (halves the 16.8MB output DMA); the last
  superblock's output DMAs alternate sync/gpsimd queues to halve the tail
  drain.
"""

import numpy as np
import ml_dtypes

S = 2048
D = 2048
H = 32
KV = 8
HD = 64
R = 4  # heads per kv group
NC = 8  # cores

BF16 = ml_dtypes.bfloat16


def _build_program():
    import concourse.mybir as mybir
    import concourse.tile as tile
    from concourse import bacc

    f32 = mybir.dt.float32
    bf16 = mybir.dt.bfloat16

    nc = bacc.Bacc("TRN2", debug=False, num_devices=NC)

    xt = nc.dram_tensor("xt", [D, S], bf16, kind="ExternalInput")
    wq_t = nc.dram_tensor("wq_t", [D, R * HD], bf16, kind="ExternalInput")
    wkv_t = nc.dram_tensor("wkv_t", [D, 2 * HD], bf16, kind="ExternalInput")
    wo_t = nc.dram_tensor("wo_t", [R * HD, D], bf16, kind="ExternalInput")
    cosb = nc.dram_tensor("cosb", [128, S], bf16, kind="ExternalInput")
    sinb = nc.dram_tensor("sinb", [128, S], bf16, kind="ExternalInput")
    tri = nc.dram_tensor("tri", [128, 128], bf16, kind="ExternalInput")
    ident64 = nc.dram_tensor("ident64", [64, 64], bf16, kind="ExternalInput")
    out_t = nc.dram_tensor("out_t", [D, S], bf16, kind="ExternalOutput")

    DT = D // 128  # 16 d tiles
    SB = S // 512  # 4 s superblocks

    with tile.TileContext(nc) as tc:
        with (
            tc.tile_pool(name="persist", bufs=1) as persist,
            tc.tile_pool(name="qstage", bufs=2) as qstage_p,
            tc.tile_pool(name="rtmp", bufs=2) as rtmp_p,
            tc.tile_pool(name="et", bufs=6) as etp,
            tc.tile_pool(name="norm", bufs=4) as normp,
            tc.tile_pool(name="ostage", bufs=3) as ostage_p,
            tc.tile_pool(name="qkv_ps", bufs=2, space="PSUM") as qkv_ps,
            tc.tile_pool(name="st_ps", bufs=2, space="PSUM") as st_ps,
            tc.tile_pool(name="ut_ps", bufs=2, space="PSUM") as ut_ps,
        ):
            # ---- persistent SBUF tensors + input DMA (ordered by first use) --
            xt_sb = persist.tile([128, DT, S], bf16)
            wq_sb = persist.tile([128, DT, R * HD], bf16)
            wkv_sb = persist.tile([128, DT, 2 * HD], bf16)
            wo_sb = persist.tile([128, 2, D], bf16)
            cos_sb = persist.tile([128, S], bf16)
            sin_sb = persist.tile([128, S], bf16)
            tri_sb = persist.tile([128, 128], bf16)
            ident_sb = persist.tile([128, 64], bf16)
            wkv_r = wkv_t.ap().rearrange("(dt p) j -> p dt j", p=128)
            wq_r = wq_t.ap().rearrange("(dt p) j -> p dt j", p=128)
            xt_r = xt.ap().rearrange("(dt p) s -> p dt s", p=128)
            # bulk activations/weights on the sync queue, ordered by first use;
            # small constants on the (otherwise idle) gpsimd queue so they
            # don't delay the xt stream.
            nc.sync.dma_start(out=wkv_sb, in_=wkv_r)
            nc.sync.dma_start(out=xt_sb[:, 0:4, 0:512], in_=xt_r[:, 0:4, 0:512])
            nc.sync.dma_start(out=xt_sb[:, 4:8, 0:512], in_=xt_r[:, 4:8, 0:512])
            nc.sync.dma_start(out=wq_sb, in_=wq_r)
            nc.sync.dma_start(out=xt_sb[:, 8:16, 0:512], in_=xt_r[:, 8:16, 0:512])
            nc.gpsimd.dma_start(out=ident_sb[64:128, :], in_=ident64.ap())
            nc.gpsimd.dma_start(out=cos_sb, in_=cosb.ap())
            nc.gpsimd.dma_start(out=sin_sb, in_=sinb.ap())
            nc.gpsimd.dma_start(out=tri_sb, in_=tri.ap())
            for sc in range(1, 4):
                nc.sync.dma_start(
                    out=xt_sb[:, :, sc * 512:(sc + 1) * 512],
                    in_=xt_r[:, :, sc * 512:(sc + 1) * 512])
            for jt in range(2):
                nc.sync.dma_start(out=wo_sb[:, jt, :], in_=wo_t.ap()[jt * 128:(jt + 1) * 128, :])

            qtr_sb = persist.tile([128, 2, S], bf16)   # roped Q, head-major
            kv2_sb = persist.tile([128, S], bf16)      # 0:64 roped K, 64:128 VT
            ko_sb = persist.tile([128, S], bf16)       # 64:128 roped K (odd heads)
            vext_sb = persist.tile([128, DT, 66], bf16)  # V blocks [t,hd] + ones col
            at_sb = persist.tile([128, 2, S], bf16)    # normalized attn out (j-major)

            # ---- PE warm-up: tiny matmuls during the DMA lead-in ----
            # keeps the HAM activity window busy so the first real matmuls run
            # at 2.4GHz instead of the cold 1.2GHz default. warm_src is
            # memset-initialized so the warm-up has no DMA dependency (input
            # DMA takes ~5us to deliver the first bytes).
            warm_src = persist.tile([64, 64], bf16)
            nc.vector.memset(warm_src, 1.0)
            warm_ps = qkv_ps.tile([64, 64], f32, tag="mm")
            for i in range(32):
                nc.tensor.matmul(
                    warm_ps, warm_src, warm_src,
                    start=(i == 0), stop=(i == 31))

            # ---- RoPE on a 512-col chunk ----
            # within each 64-row block: rows 0:32 even comps, 32:64 odd comps
            # roped = q * C + swap(q) * S  (C=[cos x4], S=[-sin,+sin]x2)
            def rope_into(dst, src, sc0, nrows, cc0):
                swp = rtmp_p.tile([128, 512], bf16, tag="swap")
                for b in range(nrows // 64):
                    nc.gpsimd.dma_start(out=swp[b * 64:b * 64 + 32, :], in_=src[b * 64 + 32:b * 64 + 64, sc0:sc0 + 512])
                    nc.gpsimd.dma_start(out=swp[b * 64 + 32:b * 64 + 64, :], in_=src[b * 64:b * 64 + 32, sc0:sc0 + 512])
                t1 = rtmp_p.tile([128, 512], bf16, tag="ropetmp")
                nc.vector.tensor_mul(t1[:nrows], src[:nrows, sc0:sc0 + 512], cos_sb[0:nrows, cc0:cc0 + 512])
                nc.vector.tensor_mul(swp[:nrows], swp[:nrows], sin_sb[0:nrows, cc0:cc0 + 512])
                nc.vector.tensor_add(dst, t1[:nrows], swp[:nrows])

            # ---- chunk projection: KV + rope + V transposes + Q + rope ----
            # emitted as a generator; each `yield` follows one PE instruction
            # so attention loops can pump it as PE filler.
            def proj_gen(c):
                # For chunk 0 (startup, nothing else on the PE) the KV and
                # first-Q matmuls interleave in dt-halves so compute tracks
                # the DMA arrival; for pumped chunks c>=1 the groups run
                # sequentially so only ONE qkv PSUM slot is held at a time,
                # leaving the second slot free for the outproj filler.
                c0, c1 = 512 * c, 512 * (c + 1)
                ps = qkv_ps.tile([128, 512], f32, tag="mm")
                if c == 0:
                    ps2 = qkv_ps.tile([128, 512], f32, tag="mm")
                    for dt in range(8):
                        nc.tensor.matmul(
                            ps, wkv_sb[:, dt, :], xt_sb[:, dt, c0:c1],
                            start=(dt == 0), stop=False)
                        yield
                    for dt in range(8):
                        nc.tensor.matmul(
                            ps2, wq_sb[:, dt, 0:128], xt_sb[:, dt, c0:c1],
                            start=(dt == 0), stop=False)
                        yield
                    for dt in range(8, DT):
                        nc.tensor.matmul(
                            ps, wkv_sb[:, dt, :], xt_sb[:, dt, c0:c1],
                            start=False, stop=(dt == DT - 1))
                        yield
                    nc.vector.tensor_copy(kv2_sb[:, c0:c1], ps)
                    rope_into(kv2_sb[0:64, c0:c1], kv2_sb, c0, 64, c0)
                    nc.gpsimd.dma_start(out=ko_sb[64:128, c0:c1], in_=kv2_sb[0:64, c0:c1])
                    for dt in range(8, DT):
                        nc.tensor.matmul(
                            ps2, wq_sb[:, dt, 0:128], xt_sb[:, dt, c0:c1],
                            start=False, stop=(dt == DT - 1))
                        yield
                else:
                    for dt in range(DT):
                        nc.tensor.matmul(
                            ps, wkv_sb[:, dt, :], xt_sb[:, dt, c0:c1],
                            start=(dt == 0), stop=(dt == DT - 1))
                        yield
                    nc.vector.tensor_copy(kv2_sb[:, c0:c1], ps)
                    rope_into(kv2_sb[0:64, c0:c1], kv2_sb, c0, 64, c0)
                    nc.gpsimd.dma_start(out=ko_sb[64:128, c0:c1], in_=kv2_sb[0:64, c0:c1])
                    ps2 = qkv_ps.tile([128, 512], f32, tag="mm")
                    for dt in range(DT):
                        nc.tensor.matmul(
                            ps2, wq_sb[:, dt, 0:128], xt_sb[:, dt, c0:c1],
                            start=(dt == 0), stop=(dt == DT - 1))
                        yield
                qst = qstage_p.tile([128, 512], bf16, tag="qstage")
                nc.vector.tensor_copy(qst, ps2)
                rope_into(qtr_sb[:, 0, c0:c1], qst, 0, 128, c0)
                for tt in range(4 * c, 4 * c + 4):
                    vps = qkv_ps.tile([128, 64], bf16, tag="mm")
                    nc.tensor.transpose(vps, kv2_sb[64:128, tt * 128:(tt + 1) * 128], ident_sb[64:128, :])
                    nc.vector.tensor_copy(vext_sb[:, tt, 0:64], vps)
                    nc.vector.memset(vext_sb[:, tt, 64:65], 1.0)
                    yield
                ps3 = qkv_ps.tile([128, 512], f32, tag="mm")
                for dt in range(DT):
                    nc.tensor.matmul(
                        ps3, wq_sb[:, dt, 128:256], xt_sb[:, dt, c0:c1],
                        start=(dt == 0), stop=(dt == DT - 1))
                    yield
                qst2 = qstage_p.tile([128, 512], bf16, tag="qstage")
                nc.vector.tensor_copy(qst2, ps3)
                rope_into(qtr_sb[:, 1, c0:c1], qst2, 0, 128, c0)
                yield

            # ---- partial out-projection of superblock so (generator) ----
            def outproj_gen(so):
                for dt in range(DT):
                    po = qkv_ps.tile([128, 512], f32, tag="mm")
                    for jt in range(2):
                        nc.tensor.matmul(
                            po, wo_sb[:, jt, dt * 128:(dt + 1) * 128],
                            at_sb[:, jt, so * 512:(so + 1) * 512],
                            start=(jt == 0), stop=(jt == 1))
                    ost = ostage_p.tile([128, 512], bf16)
                    if so == SB - 1 and dt % 2 == 0:
                        nc.scalar.activation(ost, po, mybir.ActivationFunctionType.Copy)
                    else:
                        nc.vector.tensor_copy(ost, po)
                    # split the last superblock's output DMAs across two
                    # queues so the final drain isn't serialized on one FIFO
                    dq = nc.gpsimd if (so == SB - 1 and dt % 2 == 1) else nc.sync
                    dq.dma_start(
                        out=out_t.ap()[dt * 128:(dt + 1) * 128, so * 512:(so + 1) * 512],
                        in_=ost)
                    yield

            def pump(itr, n):
                for _ in range(n):
                    try:
                        next(itr)
                    except StopIteration:
                        return

            # ---- main pipeline ----
            for _ in proj_gen(0):
                pass

            import itertools as _it
            for si in range(SB):
                nblk = 4 * (si + 1)
                c0, c1 = si * 512, (si + 1) * 512
                gens = []
                npump = 0
                if si < SB - 1:
                    gens.append(proj_gen(si + 1))
                    npump += 3 * DT + 4 + 1
                if si >= 1:
                    gens.append(outproj_gen(si - 1))
                    npump += DT
                filler = _it.chain(*gens)
                iters = 2 * nblk
                per_iter = -(-npump // iters) if npump else 0

                for jt in range(2):  # head pair (2jt, 2jt+1)
                    ut0 = ut_ps.tile([65, 512], f32, tag="ut")
                    ut1 = ut_ps.tile([65, 512], f32, tag="ut")

                    # scores for t-block j, both heads of the pair, at PE row
                    # strips 0/64 (concurrent). Diagonal blocks only cover
                    # cols >= the block start.
                    def emit_scores(j, jt=jt, c0=c0, c1=c1, si=si):
                        jj = j - 4 * si
                        s0 = 128 * jj if jj > 0 else 0
                        st2 = st_ps.tile([128, 2, 512], f32, tag="st")
                        nc.tensor.matmul(
                            st2[:, 0, s0:512],
                            kv2_sb[0:64, j * 128:(j + 1) * 128],
                            qtr_sb[0:64, jt, c0 + s0:c1], start=True, stop=True)
                        nc.tensor.matmul(
                            st2[:, 1, s0:512],
                            ko_sb[64:128, j * 128:(j + 1) * 128],
                            qtr_sb[64:128, jt, c0 + s0:c1], start=True, stop=True)
                        return st2, jj, s0

                    cur = emit_scores(0)
                    for j in range(nblk):
                        # scores(j+1) go ahead of exp(j)/AV(j) in the PE
                        # stream so ScalarE is never waiting on a filler burst
                        nxt = emit_scores(j + 1) if j + 1 < nblk else None
                        st2, jj, s0 = cur
                        et2 = etp.tile([128, 2, 512], bf16, tag="et")
                        nc.scalar.activation(
                            et2[:, :, s0:512], st2[:, :, s0:512],
                            mybir.ActivationFunctionType.Exp)
                        if jj >= 0:
                            nc.vector.tensor_mul(
                                et2[:, 0, s0:s0 + 128], et2[:, 0, s0:s0 + 128], tri_sb)
                            nc.vector.tensor_mul(
                                et2[:, 1, s0:s0 + 128], et2[:, 1, s0:s0 + 128], tri_sb)
                        # trimmed accumulation is safe: j=0 is always a full-
                        # width start=True matmul, so every PSUM column is
                        # initialized before any partial-width accumulate.
                        nc.tensor.matmul(
                            ut0[:, s0:512], vext_sb[:, j, 0:65], et2[:, 0, s0:512],
                            start=(j == 0), stop=(j == nblk - 1),
                            skip_group_check=True)
                        nc.tensor.matmul(
                            ut1[:, s0:512], vext_sb[:, j, 0:65], et2[:, 1, s0:512],
                            start=(j == 0), stop=(j == nblk - 1),
                            skip_group_check=True)
                        pump(filler, per_iter)
                        cur = nxt
                    # normalize: at = ut[0:64] / ut[64]. Evacuate the whole ut
                    # first (frees the PSUM slot immediately); the rest of the
                    # chain (denominator row to partition 0, reciprocal,
                    # gpsimd partition-broadcast, one multiply) runs off the
                    # critical path on DVE/gpsimd.
                    for half, ut in ((0, ut0), (1, ut1)):
                        utsb = normp.tile([65, 512], f32, tag="utsb")
                        nc.vector.tensor_copy(utsb, ut)
                        den0 = normp.tile([1, 512], f32, tag="den0")
                        nc.gpsimd.dma_start(out=den0, in_=utsb[64:65, :])
                        rc = normp.tile([1, 512], f32, tag="recip")
                        nc.vector.reciprocal_approx_fast(rc, den0)
                        bc = normp.tile([64, 512], f32, tag="bcast")
                        nc.gpsimd.partition_broadcast(bc, rc)
                        if half == 0:
                            nc.vector.tensor_mul(
                                at_sb[0:64, jt, c0:c1], utsb[0:64, :], bc)
                        else:
                            tmp64 = normp.tile([64, 512], bf16, tag="tmp64")
                            nc.vector.tensor_mul(tmp64, utsb[0:64, :], bc)
                            nc.gpsimd.dma_start(
                                out=at_sb[64:128, jt, c0:c1], in_=tmp64)
                # drain any leftover filler before the next superblock
                for _ in filler:
                    pass

            for _ in outproj_gen(SB - 1):
                pass

    nc.compile()
    return nc


_SIGMA = np.concatenate([np.arange(0, HD, 2), np.arange(1, HD, 2)])


def _prep_inputs(x, freqs_cis, wq, wk, wv, wo):
    """Host-side shard + layout prep. Returns per-core in_maps."""
    x = np.asarray(x, np.float32).reshape(S, D)
    freqs_cis = np.asarray(freqs_cis, np.float32)
    wq = np.asarray(wq, np.float32)
    wk = np.asarray(wk, np.float32)
    wv = np.asarray(wv, np.float32)
    wo = np.asarray(wo, np.float32)

    xt = np.ascontiguousarray(x.T).astype(BF16)

    cosT = np.ascontiguousarray(freqs_cis[:, :, 0].T)  # [32, S]
    sinT = np.ascontiguousarray(freqs_cis[:, :, 1].T)
    cosb = np.ascontiguousarray(np.tile(cosT, (4, 1))).astype(BF16)
    sinb = np.ascontiguousarray(
        np.concatenate([-sinT, sinT, -sinT, sinT], 0)).astype(BF16)

    tloc = np.arange(128)[:, None]
    sloc = np.arange(128)[None, :]
    tri = (tloc <= sloc).astype(np.float32).astype(BF16)
    ident64 = np.eye(64, dtype=np.float32).astype(BF16)

    scale = 1.0 / np.sqrt(HD)
    in_maps = []
    for g in range(NC):
        wqg = wq[g * R * HD:(g + 1) * R * HD].reshape(R, HD, D)[:, _SIGMA, :].reshape(R * HD, D)
        wq_tg = np.ascontiguousarray(wqg.T).astype(BF16)
        wkg = wk[g * HD:(g + 1) * HD][_SIGMA] * scale
        wvg = wv[g * HD:(g + 1) * HD]
        wkv_tg = np.ascontiguousarray(np.concatenate([wkg, wvg], 0).T).astype(BF16)
        wo_tg = np.ascontiguousarray(wo[:, g * R * HD:(g + 1) * R * HD].T).astype(BF16)
        in_maps.append({
            "xt": xt,
            "wq_t": wq_tg,
            "wkv_t": wkv_tg,
            "wo_t": wo_tg,
            "cosb": cosb,
            "sinb": sinb,
            "tri": tri,
            "ident64": ident64,
        })
    return in_maps


_CACHED = {}


def _get_program():
    if "nc" not in _CACHED:
        _CACHED["nc"] = _build_program()
    return _CACHED["nc"]


def kernel(x, freqs_cis, wq, wk, wv, wo, _trace=False):
    from concourse.bass_utils import run_bass_kernel_spmd

    nc = _get_program()
    in_maps = _prep_inputs(x, freqs_cis, wq, wk, wv, wo)
    res = run_bass_kernel_spmd(nc, in_maps, core_ids=list(range(NC)), trace=_trace)
    acc = np.zeros((D, S), np.float64)
    for c in range(NC):
        acc += res.results[c]["out_t"].astype(np.float64)
    out = np.ascontiguousarray(acc.T, dtype=np.float32).reshape(1, S, D)
    if _trace:
        return out, res
    return out
